# revision 27
# baseline (speedup 1.0000x reference)
"""Chamfer loss kernel for Trainium2 (8 NeuronCores, one batch per core).

Problem: B=8, N=M=8192, D=64 fp32.
  rd = pairwise euclidean distances x[b] vs y[b]   [B, N, M]
  loss = mean_b( sum_n min_m rd + sum_m min_n rd ) / M

Device strategy (per core = one batch):
  - sqrt is monotonic -> only need minima of SQUARED distances; sqrt+sums
    happen on host over 2*8192 values per batch.
  - d2 = x2 + y2 - 2*x.y is produced entirely by ONE bf16 matmul with an
    augmented contraction dim:
       lhsT rows (x side, [68, N]): [x_d (64) ; 1 ; 1 ; x2_hi ; x2_lo]
       rhs  rows (y side, [68, M]): [-2*y_d (64) ; y2_hi ; y2_lo ; 1 ; 1]
    so psum = sum_d x_d*(-2 y_d) + y2_hi + y2_lo + x2_hi + x2_lo = d2.
    (hi/lo bf16 splits keep the squared-norm terms at ~fp24 precision.)
  - Row mins (over m): DVE tensor_tensor_reduce keeps a running elementwise
    min tile AND emits the per-partition min in one pass.
  - Col mins (over n): DVE tensor_tensor min into a [128, M] accumulator
    (n folded mod 128), finished by PE transposes + DVE reduce at the end.
Host does the final sqrt / sums / mean in float64.
"""

import os

import numpy as np
import ml_dtypes

P = 128
N = 8192
D = 64
KAUG = D + 4  # 68
B = 8

_CACHE = {}


def _build_nc(n=N, mm_free=512, m_group=2048, row_mode="ttr", kaug=KAUG,
              skip_tail=False, repeat=1):
    import concourse.bass as bass
    import concourse.mybir as mybir
    import concourse.tile as tile
    from concourse import bacc
    from concourse.masks import make_identity

    fp32 = mybir.dt.float32
    bf16 = mybir.dt.bfloat16
    MIN = mybir.AluOpType.min

    nt_count = n // P          # n-tiles (output partition blocks)
    ngroups = n // m_group     # m groups per n-tile
    mm_per_g = m_group // mm_free

    # Bacc (not raw Bass): its compile pipeline lowers instructions with more
    # sync waits than the ISA's embedded slots into EventSemaphore insts.
    nc = bacc.Bacc("TRN2", target_bir_lowering=False, debug=False)
    xT = nc.dram_tensor("xT", [kaug, n], bf16, kind="ExternalInput")
    yT = nc.dram_tensor("yT", [kaug, n], bf16, kind="ExternalInput")
    out = nc.dram_tensor("out", [P, 2 * nt_count], fp32, kind="ExternalOutput")

    with tile.TileContext(nc) as tc:
        with (
            tc.tile_pool(name="const", bufs=1) as cpool,
            tc.tile_pool(name="work", bufs=3) as wpool,
            tc.tile_pool(name="psum", bufs=2, space="PSUM") as ppool,
        ):
            xTs = cpool.tile([P, n], bf16)
            yTs = cpool.tile([P, n], bf16)
            colacc = cpool.tile([P, n], bf16)
            rowacc = cpool.tile([P, m_group], bf16)
            rowmin = cpool.tile([P, nt_count], fp32)
            colmin = cpool.tile([P, nt_count], fp32)
            ident = cpool.tile([P, P], bf16)

            # chunked loads so early matmuls start before the full tensors land
            n_chunks = max(1, n // 2048)
            cw = n // n_chunks
            for c in range(n_chunks):
                nc.sync.dma_start(
                    xTs[:kaug, c * cw:(c + 1) * cw], xT[:, c * cw:(c + 1) * cw])
                nc.sync.dma_start(
                    yTs[:kaug, c * cw:(c + 1) * cw], yT[:, c * cw:(c + 1) * cw])
            make_identity(nc, ident)

            if row_mode == "tt":
                rowacc_narrow = cpool.tile([P, mm_free], bf16)
            if row_mode == "ttr2":
                rowacc2 = cpool.tile([P, m_group], bf16)

            if row_mode == "fold":
                # One n-wide s tile per n-tile: ONE wide col-min TT, and row
                # mins via a fold tree of wide TT-mins + one small reduce.
                for rep in range(repeat):
                    for nt in range(nt_count):
                        lhsT = xTs[:kaug, nt * P:(nt + 1) * P]
                        sfull = wpool.tile([P, n], bf16, tag="s",
                                           name="sfull", bufs=3)
                        for g in range(ngroups):
                            ps = ppool.tile([P, m_group], fp32,
                                            tag="ps", name="ps")
                            for k in range(mm_per_g):
                                nc.tensor.matmul(
                                    ps[:, k * mm_free:(k + 1) * mm_free],
                                    lhsT,
                                    yTs[:kaug,
                                        g * m_group + k * mm_free:
                                        g * m_group + (k + 1) * mm_free],
                                    start=True, stop=True)
                            nc.scalar.copy(
                                out=sfull[:, g * m_group:(g + 1) * m_group],
                                in_=ps)
                        if nt == 0 and rep == 0:
                            nc.vector.tensor_copy(out=colacc, in_=sfull)
                        else:
                            nc.vector.tensor_tensor(
                                out=colacc, in0=colacc, in1=sfull, op=MIN)
                        # row fold tree
                        u = wpool.tile([P, n // 2], bf16, tag="u",
                                       name="u", bufs=2)
                        nc.vector.tensor_tensor(
                            out=u, in0=sfull[:, :n // 2],
                            in1=sfull[:, n // 2:], op=MIN)
                        w = n // 2
                        while w > 512:
                            nc.vector.tensor_tensor(
                                out=u[:, :w // 2], in0=u[:, :w // 2],
                                in1=u[:, w // 2:w], op=MIN)
                            w //= 2
                        nc.vector.tensor_reduce(
                            out=rowmin[:, nt:nt + 1], in_=u[:, :w],
                            axis=mybir.AxisListType.X, op=MIN)

            for rep in range(repeat if row_mode != "fold" else 0):
              for nt in range(nt_count):
                lhsT = xTs[:kaug, nt * P:(nt + 1) * P]
                for g in range(ngroups):
                    ps = ppool.tile([P, m_group], fp32, tag="ps", name="ps")
                    for k in range(mm_per_g):
                        nc.tensor.matmul(
                            ps[:, k * mm_free:(k + 1) * mm_free],
                            lhsT,
                            yTs[:kaug, g * m_group + k * mm_free:
                                g * m_group + (k + 1) * mm_free],
                            start=True,
                            stop=True,
                        )
                    s = wpool.tile([P, m_group], bf16, name="s")
                    nc.scalar.copy(out=s, in_=ps)

                    # column-min accumulator (n folded into the 128 lanes)
                    csl = colacc[:, g * m_group:(g + 1) * m_group]
                    if nt == 0:
                        nc.vector.tensor_copy(out=csl, in_=s)
                    else:
                        nc.vector.tensor_tensor(out=csl, in0=csl, in1=s, op=MIN)

                    # row mins
                    if row_mode == "ttr2":
                        # like "ttr" but ping-pongs the elementwise-min
                        # accumulator to avoid in-place out/in1 aliasing
                        accs = [rowacc, rowacc2]
                        dst = accs[g % 2]
                        src = s if g == 0 else accs[1 - g % 2]
                        nc.vector.tensor_tensor_reduce(
                            out=dst,
                            in0=s,
                            in1=src,
                            scale=1.0,
                            scalar=3.0e38,
                            op0=MIN,
                            op1=MIN,
                            accum_out=rowmin[:, nt:nt + 1],
                        )
                    elif row_mode == "ttr":
                        # rowacc = min(rowacc, s) elementwise; accum_out gets
                        # min over the free dim of the updated rowacc. The
                        # last group's accum covers all m -> true row min.
                        nc.vector.tensor_tensor_reduce(
                            out=rowacc,
                            in0=s,
                            in1=(s if g == 0 else rowacc),
                            scale=1.0,
                            scalar=3.0e38,
                            op0=MIN,
                            op1=MIN,
                            accum_out=rowmin[:, nt:nt + 1],
                        )
                    else:
                        for k in range(mm_per_g):
                            ssl = s[:, k * mm_free:(k + 1) * mm_free]
                            if g == 0 and k == 0:
                                nc.vector.tensor_copy(out=rowacc_narrow, in_=ssl)
                            else:
                                nc.vector.tensor_tensor(
                                    out=rowacc_narrow, in0=rowacc_narrow,
                                    in1=ssl, op=MIN)
                        if g == ngroups - 1:
                            nc.vector.tensor_reduce(
                                out=rowmin[:, nt:nt + 1], in_=rowacc_narrow,
                                axis=mybir.AxisListType.X, op=MIN)

            # column-min finish: transpose each [128, 128] block of colacc on
            # PE, then min-reduce the (former partition) lanes on DVE.
            if not skip_tail:
                # batch transposes into wide bf16 PSUM tiles so the lane-min
                # runs as a few wide DVE reduces instead of nt_count small ones
                tpb = max(1, min(nt_count, (m_group * 2) // P))
                for t0 in range(0, nt_count, tpb):
                    cnt = min(tpb, nt_count - t0)
                    pt = ppool.tile([P, tpb, P], bf16, tag="ps", name="pt")
                    for i in range(cnt):
                        t = t0 + i
                        nc.tensor.transpose(
                            pt[:, i, :], colacc[:, t * P:(t + 1) * P], ident)
                    nc.vector.tensor_reduce(
                        out=colmin[:, t0:t0 + cnt], in_=pt[:, :cnt, :],
                        axis=mybir.AxisListType.X, op=MIN)
            else:
                nc.vector.tensor_copy(out=colmin, in_=rowmin)

            nc.sync.dma_start(out[:, :nt_count], rowmin[:, :])
            nc.sync.dma_start(out[:, nt_count:], colmin[:, :])

    nc.finalize()  # runs the Bacc compile passes (event sems, reg alloc, ...)
    return nc


def _prep_inputs(x, y, kaug=KAUG):
    """Build the augmented, transposed bf16 operands for each batch."""
    bf = ml_dtypes.bfloat16
    in_maps = []
    for b in range(x.shape[0]):
        xb = np.asarray(x[b], dtype=np.float32)
        yb = np.asarray(y[b], dtype=np.float32)
        n = xb.shape[0]
        x2 = np.sum(xb * xb, axis=-1)
        y2 = np.sum(yb * yb, axis=-1)
        x2_hi = x2.astype(bf)
        x2_lo = (x2 - x2_hi.astype(np.float32)).astype(bf)
        y2_hi = y2.astype(bf)
        y2_lo = (y2 - y2_hi.astype(np.float32)).astype(bf)
        ones = np.ones((1, n), dtype=bf)
        xT = np.concatenate(
            [xb.T.astype(bf), ones, ones, x2_hi[None], x2_lo[None]], axis=0)
        yT = np.concatenate(
            [(-2.0 * yb).T.astype(bf), y2_hi[None], y2_lo[None], ones, ones],
            axis=0)
        if kaug > KAUG:
            pad = np.zeros((kaug - KAUG, n), dtype=bf)
            xT = np.concatenate([xT, pad], axis=0)
            yT = np.concatenate([yT, pad], axis=0)
        in_maps.append({
            "xT": np.ascontiguousarray(xT),
            "yT": np.ascontiguousarray(yT),
        })
    return in_maps


def _postprocess(results, n=N):
    nt_count = n // P
    total = 0.0
    nb = len(results)
    for b in range(nb):
        o = np.asarray(results[b]["out"], dtype=np.float64)
        rowmin = o[:, :nt_count].T.reshape(-1)   # [n], index t*128+p
        colmin = o[:, nt_count:].T.reshape(-1)
        total += np.sqrt(np.maximum(rowmin, 0.0)).sum()
        total += np.sqrt(np.maximum(colmin, 0.0)).sum()
    loss = total / nb / n
    return np.asarray(loss, dtype=np.float32)


def _get_runner(n_cores=B):
    """Build the Bass module once and return a reusable jitted runner.

    Modeled on concourse.bass2jax.run_bass_via_pjrt's multi-core branch, but
    keeps the jitted callable so repeated invocations don't re-lower."""
    key = ("runner", n_cores)
    if key in _CACHE:
        return _CACHE[key]

    import jax
    from jax.experimental.shard_map import shard_map
    from jax.sharding import Mesh, PartitionSpec
    from concourse import bass2jax, mybir

    nc = _build_nc(row_mode=os.environ.get("CHAMFER_ROW_MODE", "tt"))

    bass2jax.install_neuronx_cc_hook()
    assert nc.dbg_addr is None

    partition_name = (
        nc.partition_id_tensor.name if nc.partition_id_tensor else None)
    in_names, out_names, out_avals = [], [], []
    for alloc in nc.m.functions[0].allocations:
        if not isinstance(alloc, mybir.MemoryLocationSet):
            continue
        name = alloc.memorylocations[0].name
        if alloc.kind == "ExternalInput":
            if name != partition_name:
                in_names.append(name)
        elif alloc.kind == "ExternalOutput":
            out_names.append(name)
            out_avals.append(jax.core.ShapedArray(
                tuple(alloc.tensor_shape), mybir.dt.np(alloc.dtype)))
    n_params = len(in_names)
    n_outs = len(out_avals)
    all_in_names = list(in_names) + list(out_names)
    if partition_name is not None:
        all_in_names.append(partition_name)
    donate = tuple(range(n_params, n_params + n_outs))

    def _body(*args):
        operands = list(args)
        if partition_name is not None:
            operands.append(bass2jax.partition_id_tensor())
        outs = bass2jax._bass_exec_p.bind(
            *operands,
            out_avals=tuple(out_avals),
            in_names=tuple(all_in_names),
            out_names=tuple(out_names),
            lowering_input_output_aliases=(),
            sim_require_finite=True,
            sim_require_nnan=True,
            nc=nc,
        )
        return tuple(outs)

    devices = jax.devices()[:n_cores]
    mesh = Mesh(np.asarray(devices), ("core",))
    sharded = jax.jit(
        shard_map(
            _body, mesh=mesh,
            in_specs=(PartitionSpec("core"),) * (n_params + n_outs),
            out_specs=(PartitionSpec("core"),) * n_outs,
            check_rep=False,
        ),
        donate_argnums=donate,
        keep_unused=True,
    )

    def run(in_maps):
        per_core = [[np.asarray(m[nm]) for nm in in_names] for m in in_maps]
        concat_in = [
            np.concatenate([per_core[c][i] for c in range(n_cores)], axis=0)
            for i in range(n_params)
        ]
        concat_zeros = [
            np.zeros((n_cores * a.shape[0], *a.shape[1:]), a.dtype)
            for a in out_avals
        ]
        out_arrs = sharded(*concat_in, *concat_zeros)
        jax.block_until_ready(out_arrs)
        return [
            {nm: np.asarray(out_arrs[i]).reshape(
                n_cores, *out_avals[i].shape)[c]
             for i, nm in enumerate(out_names)}
            for c in range(n_cores)
        ]

    _CACHE[key] = run
    return run


def kernel(x, y):
    x = np.asarray(x)
    y = np.asarray(y)
    in_maps = _prep_inputs(x, y)
    run = _get_runner(n_cores=len(in_maps))
    results = run(in_maps)
    return _postprocess(results)
